# revision 13
# baseline (speedup 1.0000x reference)
"""Mixtral-style MoE (E=8, top-2, H=1024, F=3584, T=2048) on 8 TRN2 NeuronCores.

Strategy: expert-parallel. Host computes the (tiny) router, gathers each
expert's assigned tokens (the MoE all-to-all dispatch done as input sharding),
each core runs a 3-matmul SiLU-gated MLP for ONE expert over only its routed
tokens (~4x FLOP cut vs the dense reference) in bf16, and the host
scatter-adds the 8 weighted partial outputs (the all-reduce combine done as
output unsharding).

Per-core kernel layout (all matmuls out = lhsT.T @ rhs, contraction on
partitions; token capacity C = min(512, ceil128(max tokens/expert)) so every
weight tile streams its tokens in ONE <=512-wide matmul — a single PSUM
bank — minimizing the serialized per-matmul Ldweights cost; overflow tokens
beyond the capacity are computed exactly on the host. A short stream of
garbage warm-up matmuls burns the initial DMA wait so the HAM clock gate
reaches 2.4 GHz before the real stream starts):
  phase 1: for each F-tile f (28 of 128):
           gT/uT [128f, C] = sum_k w1T[k,f].T @ xT[k, :]  (k = 8
           H-chunks of 128), PSUM-accumulated;
           actT[:, f, :] = bf16(silu(gT) * uT)      (ACT + DVE)
  phase 2: act-stationary: for each 128-token tile t (C/128 of them):
           y[t128, h] = sum_f act[f, t128].T @ w2T[f, h]  in two 512-wide
           h-slices (28 accumulating matmuls each; the 128x128 act tile is
           the stationary operand so only 112 Ldweights total); the
           PSUM->SBUF copy runs on the ACT engine as a per-partition
           (= per-token) multiply by the combine weight; DMA out y in
           [token, hidden] orientation (no host transpose).
"""

import numpy as np
import ml_dtypes

import concourse.bass as bass
import concourse.mybir as mybir
import concourse.tile as tile_mod
from concourse.tile import TileContext
from concourse.vector_clock import ScopedClock, VectorClock
from concourse.bass_utils import run_bass_kernel_spmd

E, K, H, F = 8, 2, 1024, 3584
NCORES = 8
BF16 = mybir.dt.bfloat16
F32 = mybir.dt.float32
NPBF16 = ml_dtypes.bfloat16


def _patched_drain_and_barrier(self, tick_clock, wait_clock):
    # The stock TileContext exit stacks every outstanding proc's sem wait on
    # one Drain instruction; this walrus build rejects >1 sync wait there
    # ("Too many sync wait commands"). Emit one single-wait NOP per proc on
    # the sync engine instead, then a clean drain.
    gc = tick_clock.global_clock
    n = len(gc)
    for p in range(n):
        if gc[p] > 0:
            vc = VectorClock([gc[q] if q == p else 0 for q in range(n)])
            w = self.nc.sync.nop(nofuse=True, hint="tile_exit_wait")
            wait_clock.add_sem_waits(w.ins, ScopedClock({None: vc}))
    self.nc.sync.drain()
    self.nc.all_engine_barrier()
    popped = self.nc._tile_sem_poison_stack.pop()
    assert popped is self._sem_poison
    self.nc.clear_and_free_semaphores(list(self.sems.allocated().values()))
    self.nc.all_engine_barrier()


tile_mod.TileContext._drain_and_barrier = _patched_drain_and_barrier


def _split_multi_waits(bir_json: bytes) -> bytes:
    """This walrus build rejects instructions carrying multiple sync waits.
    Hoist all-but-one wait of every instruction onto single-wait NoOps
    inserted immediately before it on the same engine (semantically identical:
    sem waits are monotonic and NX executes the stream in order)."""
    import json as _json

    bir = _json.loads(bir_json)
    ctr = 0
    for fn in bir.get("functions", []):
        for blk in fn.get("blocks", []):
            out = []
            for ins in blk.get("instructions", []):
                si = ins.get("sync_info") or {}
                w = si.get("on_wait") or []
                if len(w) > 1:
                    for extra in w[:-1]:
                        ctr += 1
                        out.append({
                            "debug": ins.get("debug", 0),
                            "engine": ins["engine"],
                            "ins": [],
                            "outs": [],
                            "name": f"I-waitsplit-{ctr}",
                            "opcode": "NoOp",
                            "sync_info": {"on_update": [], "on_wait": [extra]},
                        })
                    si["on_wait"] = [w[-1]]
                out.append(ins)
            blk["instructions"] = out
    return _json.dumps(bir).encode()


def _dedupe_ldweights(bir_json: bytes) -> bytes:
    """The bass legalizer splits every Matmult into Ldweights+Matmult pairs,
    reloading the stationary operand even when consecutive matmuls use the
    identical weights AP (the PE keeps the loaded weights until the next
    Ldweights). Drop those redundant reloads: each costs ~53ns of serialized
    PE time. A Ldweights is dropped only if its full operand signature
    matches the previous Ldweights on the same PE stream with no other
    PE instruction kinds in between, and it carries no semaphore updates;
    any waits it carries move onto the next instruction (which immediately
    followed it anyway)."""
    import json as _json

    bir = _json.loads(bir_json)
    ndropped = 0
    for fn in bir.get("functions", []):
        for blk in fn.get("blocks", []):
            out = []
            last_sig = None
            pending_waits = []
            for ins in blk.get("instructions", []):
                if ins["engine"] == "PE":
                    if ins["opcode"] == "Ldweights":
                        si = ins.get("sync_info") or {}
                        sig = _json.dumps(
                            [ins.get("ins"), ins.get("tile_position"),
                             ins.get("tile_size")], sort_keys=True)
                        if (sig == last_sig and not si.get("on_update")):
                            pending_waits.extend(si.get("on_wait") or [])
                            ndropped += 1
                            continue
                        last_sig = sig
                    elif ins["opcode"] != "Matmult":
                        last_sig = None
                if pending_waits:
                    si = ins.setdefault("sync_info",
                                        {"on_update": [], "on_wait": []})
                    si["on_wait"] = list(si.get("on_wait") or []) + pending_waits
                    pending_waits = []
                out.append(ins)
            assert not pending_waits
            blk["instructions"] = out
    return _json.dumps(bir).encode()


import concourse.bass_utils as _bass_utils_mod
import concourse.bass2jax as _bass2jax_mod

_orig_compile_bir_kernel = _bass_utils_mod.compile_bir_kernel


def _patched_compile_bir_kernel(bir_json, tmpdir, neff_name="file.neff"):
    return _orig_compile_bir_kernel(
        _split_multi_waits(_dedupe_ldweights(bir_json)), tmpdir,
        neff_name=neff_name)


_bass_utils_mod.compile_bir_kernel = _patched_compile_bir_kernel
_bass2jax_mod.compile_bir_kernel = _patched_compile_bir_kernel

# If BASS_TRACE is set but this container lacks the axon NTFF hook module,
# run_bass_kernel_spmd would crash on import. Stub it to "hook unavailable"
# so tracing degrades gracefully; a real hook, when present, is untouched.
try:
    import antenv.axon_hooks  # noqa: F401
except ImportError:
    import sys as _sys
    import types as _types
    import antenv as _antenv

    _stub = _types.ModuleType("antenv.axon_hooks")
    _stub.get_axon_ntff_profile_hook = lambda: None
    _sys.modules["antenv.axon_hooks"] = _stub
    _antenv.axon_hooks = _stub


def _route(x, gate_w):
    """Replicate the reference router in numpy fp32."""
    logits = x @ gate_w.T                                   # [T, E] f32
    m = logits.max(axis=-1, keepdims=True)
    e = np.exp(logits - m, dtype=np.float32)
    rw = e / e.sum(axis=-1, keepdims=True)                  # softmax [T, E]
    topk_idx = np.argsort(-rw, axis=-1, kind="stable")[:, :K]  # [T, K]
    topk_w = np.take_along_axis(rw, topk_idx, axis=-1)
    topk_w = topk_w / topk_w.sum(axis=-1, keepdims=True)
    return topk_idx.astype(np.int64), topk_w.astype(np.float32)


def _ceil_to(v, m):
    return -(-v // m) * m


def _build_bass(C):
    """Per-core Tile kernel at token capacity C (multiple of 4, <= 512).

    C is capped at 512 so every weight tile streams its tokens in a single
    <=512-wide matmul (one PSUM bank): 672 Ldweights+Matmult pairs per core
    instead of 1344. Tokens beyond the capacity (the few overflow slots of
    overloaded experts) are computed exactly on the host.
    """
    assert C <= 512 and C % 128 == 0
    KH = H // 128          # 8 H-chunks
    NF = F // 128          # 28 F-tiles
    NT = C // 128          # token tiles (phase-2 output partition tiles)
    NHS = H // 512         # 2 phase-2 h-slices (one PSUM bank each)

    nc = bass.Bass()
    xt_d = nc.dram_tensor("xt", [KH, 128, C], BF16, kind="ExternalInput")
    w1t_d = nc.dram_tensor("w1t", [NF, 128, KH, 128], BF16, kind="ExternalInput")
    w3t_d = nc.dram_tensor("w3t", [NF, 128, KH, 128], BF16, kind="ExternalInput")
    w2t_d = nc.dram_tensor("w2t", [NF, 128, H], BF16, kind="ExternalInput")
    wvt_d = nc.dram_tensor("wvt", [128, NT], F32, kind="ExternalInput")
    y_d = nc.dram_tensor("y", [NT, 128, H], F32, kind="ExternalOutput")

    with TileContext(nc) as tc:
        with (
            tc.tile_pool(name="resident", bufs=1) as res,
            tc.tile_pool(name="wstream", bufs=3) as wstream,
            tc.tile_pool(name="tmp", bufs=2) as tmp,
            tc.tile_pool(name="ysb", bufs=2) as ysb,
            tc.tile_pool(name="psum", bufs=2, space="PSUM") as psum,
        ):
            xt_sb = res.tile([128, KH, C], BF16, tag="xt")
            act_sb = res.tile([128, NF, C], BF16, tag="act")
            w2_sb = res.tile([128, NF, H], BF16, tag="w2")
            wvt_sb = res.tile([128, NT], F32, tag="wvt")

            # Bulk weight streaming rides the two HWDGE rings (SP + ACT):
            # HWDGE issue cost is ~0.6us/DMA on an otherwise-idle sequencer,
            # vs ~1-2.4us of Q7 emission per SWDGE (gpsimd) DMA, which at
            # this kernel's cadence would make the gpsimd ring's issue rate
            # a bottleneck. gpsimd carries only the small/early transfers
            # (xt, wvt) plus a 1/3 share of the non-critical w2 prefetch.
            # Order matters: xt feeds the very first matmul.
            dma_engines = [nc.sync, nc.scalar, nc.gpsimd]
            for k in range(KH):
                nc.gpsimd.dma_start(xt_sb[:, k, :], xt_d[k])
            nc.gpsimd.dma_start(wvt_sb[:], wvt_d[:])

            # PE warm-up: the HAM clock gate runs the PE at 1.2 GHz until it
            # has seen ~3.4us of sustained matmul activity. Burn that window
            # during the initial DMA wait with dependency-free garbage
            # matmuls (inputs uninitialized, output PSUM never read) so the
            # real stream starts at 2.4 GHz.
            warm_sb = tmp.tile([128, 512], BF16, tag="warm")
            nc.vector.memset(warm_sb[:], 0.0)
            warm_ps = psum.tile([128, C], F32, tag="g")
            n_warm = max(4, int(3600 // (C / 1.2)))
            for _ in range(n_warm):
                nc.tensor.matmul(warm_ps[:], warm_sb[:, 0:128],
                                 warm_sb[:, 0:C], start=True, stop=True)

            # ---- phase 1: gT/uT = w1/w3 contractions over H (single
            # C-wide chunk per weight tile); act = silu(g)*u in bf16
            for f in range(NF):
                w1_sb = wstream.tile([128, KH, 128], BF16, tag="w1")
                w3_sb = wstream.tile([128, KH, 128], BF16, tag="w3")
                dma_engines[f % 3].dma_start(w1_sb[:], w1t_d[f])
                dma_engines[(f + 1) % 3].dma_start(w3_sb[:], w3t_d[f])
                # stream the phase-2 weights through the third queue, lagged
                # two iterations so the first w1/w3 fetches own the early
                # HBM bandwidth (w2 isn't needed until phase 2)
                if f >= 2:
                    dma_engines[(f + 2) % 3].dma_start(
                        w2_sb[:, f - 2, :], w2t_d[f - 2])
                g_ps = psum.tile([128, C], F32, tag="g")
                u_ps = psum.tile([128, C], F32, tag="u")
                for k in range(KH):
                    nc.tensor.matmul(
                        g_ps[:], w1_sb[:, k, :], xt_sb[:, k, :],
                        start=(k == 0), stop=(k == KH - 1),
                    )
                for k in range(KH):
                    nc.tensor.matmul(
                        u_ps[:], w3_sb[:, k, :], xt_sb[:, k, :],
                        start=(k == 0), stop=(k == KH - 1),
                    )
                s_sb = tmp.tile([128, C], F32, tag="silu")
                nc.scalar.activation(
                    s_sb[:], g_ps[:], mybir.ActivationFunctionType.Silu
                )
                nc.vector.tensor_tensor(
                    act_sb[:, f, :], s_sb[:], u_ps[:],
                    mybir.AluOpType.mult,
                )
            for f in range(NF - 2, NF):
                dma_engines[(f + 2) % 3].dma_start(w2_sb[:, f, :], w2t_d[f])

            # ---- phase 2: act-stationary. y[t, h] = sum_f act[f, t].T @
            # w2T[f, h]: the 128x128 act tile is the stationary operand
            # (112 Ldweights instead of 224) and w2 streams as the moving
            # operand in two 512-wide h-slices (one PSUM bank each). The
            # per-token combine weight is applied by the ACT engine as a
            # per-partition scale during the PSUM->SBUF copy, so the output
            # leaves in [token, hidden] orientation (no host transpose).
            for t in range(NT):
                y0_ps = psum.tile([128, 512], F32, tag="y0")
                y1_ps = psum.tile([128, 512], F32, tag="y1")
                for f in range(NF):
                    a_t = act_sb[:, f, t * 128:(t + 1) * 128]
                    nc.tensor.matmul(
                        y0_ps[:], a_t, w2_sb[:, f, 0:512],
                        start=(f == 0), stop=(f == NF - 1),
                    )
                    nc.tensor.matmul(
                        y1_ps[:], a_t, w2_sb[:, f, 512:1024],
                        start=(f == 0), stop=(f == NF - 1),
                    )
                y_sb = ysb.tile([128, H], F32, tag="y")
                wv_col = wvt_sb[:, t:t + 1]
                nc.scalar.mul(y_sb[:, 0:512], y0_ps[:], wv_col)
                nc.scalar.mul(y_sb[:, 512:1024], y1_ps[:], wv_col)
                if t == NT - 1:
                    # split the last tile across both HWDGE rings so its
                    # completion latencies overlap
                    nc.sync.dma_start(y_d[t, :, 0:512], y_sb[:, 0:512])
                    nc.scalar.dma_start(y_d[t, :, 512:1024], y_sb[:, 512:1024])
                else:
                    dma_engines[t % 3].dma_start(y_d[t], y_sb[:])

    return nc


def kernel(hidden_states, gate_w, w1, w3, w2):
    x = np.ascontiguousarray(np.asarray(hidden_states, np.float32)).reshape(-1, H)
    gate_w = np.asarray(gate_w, np.float32)
    w1 = np.asarray(w1, np.float32)
    w3 = np.asarray(w3, np.float32)
    w2 = np.asarray(w2, np.float32)
    T = x.shape[0]

    topk_idx, topk_w = _route(x, gate_w)

    idx_e, wv_e = [], []
    for e in range(E):
        sel_t, sel_k = np.nonzero(topk_idx == e)
        idx_e.append(sel_t)
        wv_e.append(topk_w[sel_t, sel_k])
    maxT = max(len(i) for i in idx_e)
    # Device capacity: <=512 tokens per expert (single PSUM-bank-wide matmul
    # chunks; multiple of 128 for the phase-2 token tiles). Overflow slots of
    # overloaded experts run on the host below.
    C = max(128, min(_ceil_to(maxT, 128), 512))

    xbf = x.astype(NPBF16)
    in_maps = []
    for e in range(E):
        n = min(len(idx_e[e]), C)
        xg = np.zeros((C, H), NPBF16)
        xg[:n] = xbf[idx_e[e][:n]]
        xt = np.ascontiguousarray(xg.T).reshape(H // 128, 128, C)
        w1t = np.ascontiguousarray(
            w1[e].astype(NPBF16).reshape(F // 128, 128, H // 128, 128)
            .transpose(0, 3, 2, 1)
        )
        w3t = np.ascontiguousarray(
            w3[e].astype(NPBF16).reshape(F // 128, 128, H // 128, 128)
            .transpose(0, 3, 2, 1)
        )
        w2t = np.ascontiguousarray(w2[e].T.astype(NPBF16)).reshape(F // 128, 128, H)
        wv = np.zeros(C, np.float32)
        wv[:n] = wv_e[e][:n]
        # [128, NT]: partition p, column t -> combine weight of token 128t+p
        wvt = np.ascontiguousarray(wv.reshape(C // 128, 128).T)
        in_maps.append({"xt": xt, "w1t": w1t, "w3t": w3t, "w2t": w2t,
                        "wvt": wvt})

    nc = _build_bass(C)
    res = run_bass_kernel_spmd(nc, in_maps, core_ids=list(range(NCORES)))
    global last_results, last_in_maps, last_C
    last_results, last_in_maps, last_C = res, in_maps, C

    out = np.zeros((T, H), np.float32)
    for e in range(E):
        n = min(len(idx_e[e]), C)
        yt = res.results[e]["y"].reshape(C, H)   # [C, H], token-major
        out[idx_e[e][:n]] += yt[:n]
        if len(idx_e[e]) > C:
            # Capacity overflow: exact host-side SiLU MLP for the few
            # leftover token slots of this expert.
            ov = idx_e[e][C:]
            xo = x[ov]                              # [m, H] f32
            g = xo @ w1[e].T
            u = xo @ w3[e].T
            act = (g / (1.0 + np.exp(-g))) * u
            yo = act @ w2[e].T
            out[ov] += wv_e[e][C:, None] * yo
    return out.reshape(1, T, H).astype(np.float32)



# revision 25
# speedup vs baseline: 1.4775x; 1.4775x over previous
"""Mixtral-style MoE (E=8, top-2, H=1024, F=3584, T=2048) on 8 TRN2 NeuronCores.

Strategy: expert-parallel. Host computes the (tiny) router, gathers each
expert's assigned tokens (the MoE all-to-all dispatch done as input sharding),
each core runs a 3-matmul SiLU-gated MLP for ONE expert over only its routed
tokens (~4x FLOP cut vs the dense reference) in bf16, and the host
scatter-adds the 8 weighted partial outputs (the all-reduce combine done as
output unsharding).

Per-core kernel layout (all matmuls out = lhsT.T @ rhs, contraction on
partitions; token capacity C = min(512, ceil128(max tokens/expert)) so every
weight tile streams its tokens in ONE <=512-wide matmul — a single PSUM
bank — minimizing the serialized per-matmul Ldweights cost; overflow tokens
beyond the capacity are computed exactly on the host. A short stream of
garbage warm-up matmuls burns the initial DMA wait so the HAM clock gate
reaches 2.4 GHz before the real stream starts):
  phase 1: for each F-tile f (28 of 128):
           gT/uT [128f, C] = sum_k w1T[k,f].T @ xT[k, :]  (k = 8
           H-chunks of 128), PSUM-accumulated;
           actT[:, f, :] = bf16(silu(gT) * uT)      (ACT + DVE)
  phase 2: act-stationary: for each 128-token tile t (C/128 of them):
           y[t128, h] = sum_f act[f, t128].T @ w2T[f, h]  in two 512-wide
           h-slices (28 accumulating matmuls each; the 128x128 act tile is
           the stationary operand so only 112 Ldweights total); the
           PSUM->SBUF copy runs on the ACT engine as a per-partition
           (= per-token) multiply by the combine weight; DMA out y in
           [token, hidden] orientation (no host transpose).
"""

import numpy as np
import ml_dtypes

import concourse.bass as bass
import concourse.mybir as mybir
import concourse.tile as tile_mod
from concourse.tile import TileContext
from concourse.vector_clock import ScopedClock, VectorClock
from concourse.bass_utils import run_bass_kernel_spmd

E, K, H, F = 8, 2, 1024, 3584
NCORES = 8
BF16 = mybir.dt.bfloat16
F32 = mybir.dt.float32
NPBF16 = ml_dtypes.bfloat16


def _patched_drain_and_barrier(self, tick_clock, wait_clock):
    # The stock TileContext exit stacks every outstanding proc's sem wait on
    # one Drain instruction; this walrus build rejects >1 sync wait there
    # ("Too many sync wait commands"). Emit one single-wait NOP per proc on
    # the sync engine instead, then a clean drain.
    gc = tick_clock.global_clock
    n = len(gc)
    for p in range(n):
        if gc[p] > 0:
            vc = VectorClock([gc[q] if q == p else 0 for q in range(n)])
            w = self.nc.sync.nop(nofuse=True, hint="tile_exit_wait")
            wait_clock.add_sem_waits(w.ins, ScopedClock({None: vc}))
    self.nc.sync.drain()
    self.nc.all_engine_barrier()
    popped = self.nc._tile_sem_poison_stack.pop()
    assert popped is self._sem_poison
    self.nc.clear_and_free_semaphores(list(self.sems.allocated().values()))
    self.nc.all_engine_barrier()


tile_mod.TileContext._drain_and_barrier = _patched_drain_and_barrier


def _split_multi_waits(bir_json: bytes) -> bytes:
    """This walrus build rejects instructions carrying multiple sync waits.
    Hoist all-but-one wait of every instruction onto single-wait NoOps
    inserted immediately before it on the same engine (semantically identical:
    sem waits are monotonic and NX executes the stream in order)."""
    import json as _json

    bir = _json.loads(bir_json)
    ctr = 0
    for fn in bir.get("functions", []):
        for blk in fn.get("blocks", []):
            out = []
            for ins in blk.get("instructions", []):
                si = ins.get("sync_info") or {}
                w = si.get("on_wait") or []
                if len(w) > 1:
                    for extra in w[:-1]:
                        ctr += 1
                        out.append({
                            "debug": ins.get("debug", 0),
                            "engine": ins["engine"],
                            "ins": [],
                            "outs": [],
                            "name": f"I-waitsplit-{ctr}",
                            "opcode": "NoOp",
                            "sync_info": {"on_update": [], "on_wait": [extra]},
                        })
                    si["on_wait"] = [w[-1]]
                out.append(ins)
            blk["instructions"] = out
    return _json.dumps(bir).encode()


def _dedupe_ldweights(bir_json: bytes) -> bytes:
    """The bass legalizer splits every Matmult into Ldweights+Matmult pairs,
    reloading the stationary operand even when consecutive matmuls use the
    identical weights AP (the PE keeps the loaded weights until the next
    Ldweights). Drop those redundant reloads: each costs ~53ns of serialized
    PE time. A Ldweights is dropped only if its full operand signature
    matches the previous Ldweights on the same PE stream with no other
    PE instruction kinds in between, and it carries no semaphore updates;
    any waits it carries move onto the next instruction (which immediately
    followed it anyway)."""
    import json as _json

    bir = _json.loads(bir_json)
    ndropped = 0
    for fn in bir.get("functions", []):
        for blk in fn.get("blocks", []):
            out = []
            last_sig = None
            pending_waits = []
            for ins in blk.get("instructions", []):
                if ins["engine"] == "PE":
                    if ins["opcode"] == "Ldweights":
                        si = ins.get("sync_info") or {}
                        sig = _json.dumps(
                            [ins.get("ins"), ins.get("tile_position"),
                             ins.get("tile_size")], sort_keys=True)
                        if (sig == last_sig and not si.get("on_update")):
                            pending_waits.extend(si.get("on_wait") or [])
                            ndropped += 1
                            continue
                        last_sig = sig
                    elif ins["opcode"] != "Matmult":
                        last_sig = None
                if pending_waits:
                    si = ins.setdefault("sync_info",
                                        {"on_update": [], "on_wait": []})
                    si["on_wait"] = list(si.get("on_wait") or []) + pending_waits
                    pending_waits = []
                out.append(ins)
            assert not pending_waits
            blk["instructions"] = out
    return _json.dumps(bir).encode()


import concourse.bass_utils as _bass_utils_mod
import concourse.bass2jax as _bass2jax_mod

_orig_compile_bir_kernel = _bass_utils_mod.compile_bir_kernel


def _patched_compile_bir_kernel(bir_json, tmpdir, neff_name="file.neff"):
    return _orig_compile_bir_kernel(
        _split_multi_waits(_dedupe_ldweights(bir_json)), tmpdir,
        neff_name=neff_name)


_bass_utils_mod.compile_bir_kernel = _patched_compile_bir_kernel
_bass2jax_mod.compile_bir_kernel = _patched_compile_bir_kernel

# If BASS_TRACE is set but this container lacks the axon NTFF hook module,
# run_bass_kernel_spmd would crash on import. Stub it to "hook unavailable"
# so tracing degrades gracefully; a real hook, when present, is untouched.
try:
    import antenv.axon_hooks  # noqa: F401
except ImportError:
    import sys as _sys
    import types as _types
    import antenv as _antenv

    _stub = _types.ModuleType("antenv.axon_hooks")
    _stub.get_axon_ntff_profile_hook = lambda: None
    _sys.modules["antenv.axon_hooks"] = _stub
    _antenv.axon_hooks = _stub


def _route(x, gate_w):
    """Replicate the reference router in numpy fp32."""
    logits = x @ gate_w.T                                   # [T, E] f32
    m = logits.max(axis=-1, keepdims=True)
    e = np.exp(logits - m, dtype=np.float32)
    rw = e / e.sum(axis=-1, keepdims=True)                  # softmax [T, E]
    topk_idx = np.argsort(-rw, axis=-1, kind="stable")[:, :K]  # [T, K]
    topk_w = np.take_along_axis(rw, topk_idx, axis=-1)
    topk_w = topk_w / topk_w.sum(axis=-1, keepdims=True)
    return topk_idx.astype(np.int64), topk_w.astype(np.float32)


def _ceil_to(v, m):
    return -(-v // m) * m


def _build_bass(C):
    """Per-core Tile kernel at token capacity C (multiple of 4, <= 512).

    C is capped at 512 so every weight tile streams its tokens in a single
    <=512-wide matmul (one PSUM bank): 672 Ldweights+Matmult pairs per core
    instead of 1344. Tokens beyond the capacity (the few overflow slots of
    overloaded experts) are computed exactly on the host.
    """
    assert C <= 512 and C % 128 == 0
    KH = H // 128          # 8 H-chunks
    NF = F // 128          # 28 F-tiles
    NT = C // 128          # token tiles (phase-2 output partition tiles)

    nc = bass.Bass()
    xt_d = nc.dram_tensor("xt", [KH, 128, C], BF16, kind="ExternalInput")
    w1t_d = nc.dram_tensor("w1t", [NF, 128, KH, 128], BF16, kind="ExternalInput")
    w3t_d = nc.dram_tensor("w3t", [NF, 128, KH, 128], BF16, kind="ExternalInput")
    w2t_d = nc.dram_tensor("w2t", [NF, 128, H], BF16, kind="ExternalInput")
    wvt_d = nc.dram_tensor("wvt", [128, NT], F32, kind="ExternalInput")
    # y in bf16: output quantization error (~0.2% of a value that itself
    # carries ~0.4% bf16 matmul noise) is far inside the accuracy budget,
    # and it halves the output DMA.
    y_d = nc.dram_tensor("y", [NT, 128, H], BF16, kind="ExternalOutput")

    with TileContext(nc) as tc:
        with (
            tc.tile_pool(name="resident", bufs=1) as res,
            tc.tile_pool(name="wstream", bufs=3) as wstream,
            tc.tile_pool(name="tmp", bufs=2) as tmp,
            tc.tile_pool(name="ysb", bufs=2) as ysb,
            tc.tile_pool(name="psum", bufs=2, space="PSUM") as psum,
        ):
            xt_sb = res.tile([128, KH, C], BF16, tag="xt")
            act_sb = res.tile([128, NF, C], BF16, tag="act")
            w2_sb = res.tile([128, NF, H], BF16, tag="w2")
            wvt_sb = res.tile([128, NT], F32, tag="wvt")

            # All bulk streaming rides the two HWDGE rings (SP + ACT): HWDGE
            # issue cost is ~0.6us/DMA on an otherwise-idle sequencer, vs
            # ~1-2.4us of Q7 emission per SWDGE (gpsimd) DMA — at this
            # kernel's ~2.1us/f-tile cadence a gpsimd ring carrying a third
            # of the weight stream would be issue-rate-limited. The 16 SDMA
            # engines (and the ~358 GB/s HBM ceiling) are shared across
            # rings, so two rings lose no bandwidth. gpsimd carries only the
            # tiny wvt load. Order matters: xt feeds the very first matmul,
            # so its 8 chunks go 4/4 on both rings ahead of any weights.
            hwdge = [nc.sync, nc.scalar]
            for k in range(KH):
                hwdge[k % 2].dma_start(xt_sb[:, k, :], xt_d[k])
            nc.gpsimd.dma_start(wvt_sb[:], wvt_d[:])

            # PE warm-up: the HAM clock gate runs the PE at 1.2 GHz until it
            # has seen ~3.4us of sustained matmul activity. Burn that window
            # during the initial DMA wait with dependency-free garbage
            # matmuls (the PSUM bank is overwritten by the first real
            # accumulation's start=True). The warm tile is kept to a single
            # 128-wide block so its memset barely delays the first matmul.
            warm_sb = tmp.tile([128, 128], BF16, tag="warm")
            nc.vector.memset(warm_sb[:], 0.0)
            warm_ps = psum.tile([128, 128], F32, tag="g")
            n_warm = max(4, int(3600 // (128 / 1.2)))
            for _ in range(n_warm):
                nc.tensor.matmul(warm_ps[:], warm_sb[:],
                                 warm_sb[:], start=True, stop=True)

            # ---- phase 1: gT/uT = w1/w3 contractions over H (single
            # C-wide chunk per weight tile); act = silu(g)*u in bf16
            for f in range(NF):
                w1_sb = wstream.tile([128, KH, 128], BF16, tag="w1")
                w3_sb = wstream.tile([128, KH, 128], BF16, tag="w3")
                hwdge[f % 2].dma_start(w1_sb[:], w1t_d[f])
                hwdge[(f + 1) % 2].dma_start(w3_sb[:], w3t_d[f])
                # stream the phase-2 weights on the alternating ring, lagged
                # two iterations so the first w1/w3 fetches own the early
                # HBM bandwidth. Only the first 512-wide h-slice is part of
                # the phase-1 critical window: phase 2 touches w2[:, 512:]
                # a few us in, so that half is deferred below, shrinking the
                # DMA-bound phase-1 span by ~3.6 MB.
                if f >= 2:
                    hwdge[f % 2].dma_start(w2_sb[:, f - 2, 0:512],
                                           w2t_d[f - 2, :, 0:512])
                g_ps = psum.tile([128, C], F32, tag="g")
                u_ps = psum.tile([128, C], F32, tag="u")
                for k in range(KH):
                    nc.tensor.matmul(
                        g_ps[:], w1_sb[:, k, :], xt_sb[:, k, :],
                        start=(k == 0), stop=(k == KH - 1),
                    )
                for k in range(KH):
                    nc.tensor.matmul(
                        u_ps[:], w3_sb[:, k, :], xt_sb[:, k, :],
                        start=(k == 0), stop=(k == KH - 1),
                    )
                s_sb = tmp.tile([128, C], F32, tag="silu")
                nc.scalar.activation(
                    s_sb[:], g_ps[:], mybir.ActivationFunctionType.Silu
                )
                nc.vector.tensor_tensor(
                    act_sb[:, f, :], s_sb[:], u_ps[:],
                    mybir.AluOpType.mult,
                )
            for f in range(NF - 2, NF):
                hwdge[f % 2].dma_start(w2_sb[:, f, 0:512], w2t_d[f, :, 0:512])
            # deferred second h-slice of w2: consumed by phase 2's y1
            # matmuls, which start ~6us after phase 2 begins
            for f in range(NF):
                hwdge[f % 2].dma_start(w2_sb[:, f, 512:1024],
                                       w2t_d[f, :, 512:1024])

            # ---- phase 2: act-stationary. y[t, h] = sum_f act[f, t].T @
            # w2T[f, h]: the 128x128 act tile is the stationary operand
            # (112 Ldweights instead of 224) and w2 streams as the moving
            # operand in two 512-wide h-slices (one PSUM bank each). The
            # per-token combine weight is applied by the ACT engine as a
            # per-partition scale during the PSUM->SBUF copy, so the output
            # leaves in [token, hidden] orientation (no host transpose).
            for t in range(NT):
                y0_ps = psum.tile([128, 512], F32, tag="y0")
                y1_ps = psum.tile([128, 512], F32, tag="y1")
                for f in range(NF):
                    a_t = act_sb[:, f, t * 128:(t + 1) * 128]
                    nc.tensor.matmul(
                        y0_ps[:], a_t, w2_sb[:, f, 0:512],
                        start=(f == 0), stop=(f == NF - 1),
                    )
                    nc.tensor.matmul(
                        y1_ps[:], a_t, w2_sb[:, f, 512:1024],
                        start=(f == 0), stop=(f == NF - 1),
                    )
                y_sb = ysb.tile([128, H], BF16, tag="y")
                wv_col = wvt_sb[:, t:t + 1]
                # drain the two banks on different engines so they run in
                # parallel (matters for the final tile's exposed tail)
                nc.scalar.mul(y_sb[:, 0:512], y0_ps[:], wv_col)
                nc.vector.tensor_scalar_mul(y_sb[:, 512:1024], y1_ps[:],
                                            wv_col)
                if t == NT - 1:
                    # split the last tile across both HWDGE rings so its
                    # completion latencies overlap
                    nc.sync.dma_start(y_d[t, :, 0:512], y_sb[:, 0:512])
                    nc.scalar.dma_start(y_d[t, :, 512:1024], y_sb[:, 512:1024])
                else:
                    hwdge[t % 2].dma_start(y_d[t], y_sb[:])

    return nc


def kernel(hidden_states, gate_w, w1, w3, w2):
    x = np.ascontiguousarray(np.asarray(hidden_states, np.float32)).reshape(-1, H)
    gate_w = np.asarray(gate_w, np.float32)
    w1 = np.asarray(w1, np.float32)
    w3 = np.asarray(w3, np.float32)
    w2 = np.asarray(w2, np.float32)
    T = x.shape[0]

    topk_idx, topk_w = _route(x, gate_w)

    idx_e, wv_e = [], []
    for e in range(E):
        sel_t, sel_k = np.nonzero(topk_idx == e)
        idx_e.append(sel_t)
        wv_e.append(topk_w[sel_t, sel_k])
    maxT = max(len(i) for i in idx_e)
    # Device capacity: <=512 tokens per expert (single PSUM-bank-wide matmul
    # chunks; multiple of 128 for the phase-2 token tiles). Overflow slots of
    # overloaded experts run on the host below.
    C = max(128, min(_ceil_to(maxT, 128), 512))

    xbf = x.astype(NPBF16)
    in_maps = []
    for e in range(E):
        n = min(len(idx_e[e]), C)
        xg = np.zeros((C, H), NPBF16)
        xg[:n] = xbf[idx_e[e][:n]]
        xt = np.ascontiguousarray(xg.T).reshape(H // 128, 128, C)
        w1t = np.ascontiguousarray(
            w1[e].astype(NPBF16).reshape(F // 128, 128, H // 128, 128)
            .transpose(0, 3, 2, 1)
        )
        w3t = np.ascontiguousarray(
            w3[e].astype(NPBF16).reshape(F // 128, 128, H // 128, 128)
            .transpose(0, 3, 2, 1)
        )
        w2t = np.ascontiguousarray(w2[e].T.astype(NPBF16)).reshape(F // 128, 128, H)
        wv = np.zeros(C, np.float32)
        wv[:n] = wv_e[e][:n]
        # [128, NT]: partition p, column t -> combine weight of token 128t+p
        wvt = np.ascontiguousarray(wv.reshape(C // 128, 128).T)
        in_maps.append({"xt": xt, "w1t": w1t, "w3t": w3t, "w2t": w2t,
                        "wvt": wvt})

    nc = _build_bass(C)
    res = run_bass_kernel_spmd(nc, in_maps, core_ids=list(range(NCORES)))
    global last_results, last_in_maps, last_C
    last_results, last_in_maps, last_C = res, in_maps, C

    out = np.zeros((T, H), np.float32)
    for e in range(E):
        n = min(len(idx_e[e]), C)
        yt = res.results[e]["y"].reshape(C, H)   # [C, H] bf16, token-major
        out[idx_e[e][:n]] += yt[:n].astype(np.float32)
        if len(idx_e[e]) > C:
            # Capacity overflow: exact host-side SiLU MLP for the few
            # leftover token slots of this expert.
            ov = idx_e[e][C:]
            xo = x[ov]                              # [m, H] f32
            g = xo @ w1[e].T
            u = xo @ w3[e].T
            act = (g / (1.0 + np.exp(-g))) * u
            yo = act @ w2[e].T
            out[ov] += wv_e[e][C:, None] * yo
    return out.reshape(1, T, H).astype(np.float32)



# revision 27
# speedup vs baseline: 3.8496x; 2.6054x over previous
"""Mixtral-style MoE (E=8, top-2, H=1024, F=3584, T=2048) on 8 TRN2 NeuronCores.

Strategy: expert-parallel. Host computes the (tiny) router, gathers each
expert's assigned tokens (the MoE all-to-all dispatch done as input sharding),
each core runs a 3-matmul SiLU-gated MLP for ONE expert over only its routed
tokens (~4x FLOP cut vs the dense reference) in bf16, and the host
scatter-adds the 8 weighted partial outputs (the all-reduce combine done as
output unsharding).

Per-core kernel layout (all matmuls out = lhsT.T @ rhs, contraction on
partitions; token capacity C = min(512, ceil128(max tokens/expert)) so every
weight tile streams its tokens in ONE <=512-wide matmul — a single PSUM
bank — minimizing the serialized per-matmul Ldweights cost; overflow tokens
beyond the capacity are computed exactly on the host. A short stream of
garbage warm-up matmuls burns the initial DMA wait so the HAM clock gate
reaches 2.4 GHz before the real stream starts):
  phase 1: for each F-tile f (28 of 128):
           gT/uT [128f, C] = sum_k w1T[k,f].T @ xT[k, :]  (k = 8
           H-chunks of 128), PSUM-accumulated;
           actT[:, f, :] = bf16(silu(gT) * uT)      (ACT + DVE)
  phase 2: act-stationary: for each 128-token tile t (C/128 of them):
           y[t128, h] = sum_f act[f, t128].T @ w2T[f, h]  in two 512-wide
           h-slices (28 accumulating matmuls each; the 128x128 act tile is
           the stationary operand so only 112 Ldweights total); the
           PSUM->SBUF copy runs on the ACT engine as a per-partition
           (= per-token) multiply by the combine weight; DMA out y in
           [token, hidden] orientation (no host transpose).
"""

import numpy as np
import ml_dtypes

import concourse.bass as bass
import concourse.mybir as mybir
import concourse.tile as tile_mod
from concourse.tile import TileContext
from concourse.vector_clock import ScopedClock, VectorClock
from concourse.bass_utils import run_bass_kernel_spmd

E, K, H, F = 8, 2, 1024, 3584
NCORES = 8
BF16 = mybir.dt.bfloat16
F32 = mybir.dt.float32
NPBF16 = ml_dtypes.bfloat16


def _patched_drain_and_barrier(self, tick_clock, wait_clock):
    # The stock TileContext exit stacks every outstanding proc's sem wait on
    # one Drain instruction; this walrus build rejects >1 sync wait there
    # ("Too many sync wait commands"). Emit one single-wait NOP per proc on
    # the sync engine instead, then a clean drain.
    gc = tick_clock.global_clock
    n = len(gc)
    for p in range(n):
        if gc[p] > 0:
            vc = VectorClock([gc[q] if q == p else 0 for q in range(n)])
            w = self.nc.sync.nop(nofuse=True, hint="tile_exit_wait")
            wait_clock.add_sem_waits(w.ins, ScopedClock({None: vc}))
    self.nc.sync.drain()
    self.nc.all_engine_barrier()
    popped = self.nc._tile_sem_poison_stack.pop()
    assert popped is self._sem_poison
    self.nc.clear_and_free_semaphores(list(self.sems.allocated().values()))
    self.nc.all_engine_barrier()


tile_mod.TileContext._drain_and_barrier = _patched_drain_and_barrier


def _split_multi_waits(bir_json: bytes) -> bytes:
    """This walrus build rejects instructions carrying multiple sync waits.
    Hoist all-but-one wait of every instruction onto single-wait NoOps
    inserted immediately before it on the same engine (semantically identical:
    sem waits are monotonic and NX executes the stream in order)."""
    import json as _json

    bir = _json.loads(bir_json)
    ctr = 0
    for fn in bir.get("functions", []):
        for blk in fn.get("blocks", []):
            out = []
            for ins in blk.get("instructions", []):
                si = ins.get("sync_info") or {}
                w = si.get("on_wait") or []
                if len(w) > 1:
                    for extra in w[:-1]:
                        ctr += 1
                        out.append({
                            "debug": ins.get("debug", 0),
                            "engine": ins["engine"],
                            "ins": [],
                            "outs": [],
                            "name": f"I-waitsplit-{ctr}",
                            "opcode": "NoOp",
                            "sync_info": {"on_update": [], "on_wait": [extra]},
                        })
                    si["on_wait"] = [w[-1]]
                out.append(ins)
            blk["instructions"] = out
    return _json.dumps(bir).encode()


def _dedupe_ldweights(bir_json: bytes) -> bytes:
    """The bass legalizer splits every Matmult into Ldweights+Matmult pairs,
    reloading the stationary operand even when consecutive matmuls use the
    identical weights AP (the PE keeps the loaded weights until the next
    Ldweights). Drop those redundant reloads: each costs ~53ns of serialized
    PE time. A Ldweights is dropped only if its full operand signature
    matches the previous Ldweights on the same PE stream with no other
    PE instruction kinds in between, and it carries no semaphore updates;
    any waits it carries move onto the next instruction (which immediately
    followed it anyway)."""
    import json as _json

    bir = _json.loads(bir_json)
    ndropped = 0
    for fn in bir.get("functions", []):
        for blk in fn.get("blocks", []):
            out = []
            last_sig = None
            pending_waits = []
            for ins in blk.get("instructions", []):
                if ins["engine"] == "PE":
                    if ins["opcode"] == "Ldweights":
                        si = ins.get("sync_info") or {}
                        sig = _json.dumps(
                            [ins.get("ins"), ins.get("tile_position"),
                             ins.get("tile_size")], sort_keys=True)
                        if (sig == last_sig and not si.get("on_update")):
                            pending_waits.extend(si.get("on_wait") or [])
                            ndropped += 1
                            continue
                        last_sig = sig
                    elif ins["opcode"] != "Matmult":
                        last_sig = None
                if pending_waits:
                    si = ins.setdefault("sync_info",
                                        {"on_update": [], "on_wait": []})
                    si["on_wait"] = list(si.get("on_wait") or []) + pending_waits
                    pending_waits = []
                out.append(ins)
            assert not pending_waits
            blk["instructions"] = out
    return _json.dumps(bir).encode()


import concourse.bass_utils as _bass_utils_mod
import concourse.bass2jax as _bass2jax_mod

_orig_compile_bir_kernel = _bass_utils_mod.compile_bir_kernel


def _patched_compile_bir_kernel(bir_json, tmpdir, neff_name="file.neff"):
    return _orig_compile_bir_kernel(
        _split_multi_waits(_dedupe_ldweights(bir_json)), tmpdir,
        neff_name=neff_name)


_bass_utils_mod.compile_bir_kernel = _patched_compile_bir_kernel
_bass2jax_mod.compile_bir_kernel = _patched_compile_bir_kernel

# If BASS_TRACE is set but this container lacks the axon NTFF hook module,
# run_bass_kernel_spmd would crash on import. Stub it to "hook unavailable"
# so tracing degrades gracefully; a real hook, when present, is untouched.
try:
    import antenv.axon_hooks  # noqa: F401
except ImportError:
    import sys as _sys
    import types as _types
    import antenv as _antenv

    _stub = _types.ModuleType("antenv.axon_hooks")
    _stub.get_axon_ntff_profile_hook = lambda: None
    _sys.modules["antenv.axon_hooks"] = _stub
    _antenv.axon_hooks = _stub


def _route(x, gate_w):
    """Replicate the reference router in numpy fp32."""
    logits = x @ gate_w.T                                   # [T, E] f32
    m = logits.max(axis=-1, keepdims=True)
    e = np.exp(logits - m, dtype=np.float32)
    rw = e / e.sum(axis=-1, keepdims=True)                  # softmax [T, E]
    topk_idx = np.argsort(-rw, axis=-1, kind="stable")[:, :K]  # [T, K]
    topk_w = np.take_along_axis(rw, topk_idx, axis=-1)
    topk_w = topk_w / topk_w.sum(axis=-1, keepdims=True)
    return topk_idx.astype(np.int64), topk_w.astype(np.float32)


def _ceil_to(v, m):
    return -(-v // m) * m


def _build_bass(C):
    """Per-core Tile kernel at token capacity C (multiple of 4, <= 512).

    C is capped at 512 so every weight tile streams its tokens in a single
    <=512-wide matmul (one PSUM bank): 672 Ldweights+Matmult pairs per core
    instead of 1344. Tokens beyond the capacity (the few overflow slots of
    overloaded experts) are computed exactly on the host.
    """
    assert C <= 512 and C % 128 == 0
    KH = H // 128          # 8 H-chunks
    NF = F // 128          # 28 F-tiles
    NT = C // 128          # token tiles (phase-2 output partition tiles)

    nc = bass.Bass()
    xt_d = nc.dram_tensor("xt", [KH, 128, C], BF16, kind="ExternalInput")
    w1t_d = nc.dram_tensor("w1t", [NF, 128, KH, 128], BF16, kind="ExternalInput")
    w3t_d = nc.dram_tensor("w3t", [NF, 128, KH, 128], BF16, kind="ExternalInput")
    w2t_d = nc.dram_tensor("w2t", [NF, 128, H], BF16, kind="ExternalInput")
    wvt_d = nc.dram_tensor("wvt", [128, NT], F32, kind="ExternalInput")
    # y in bf16: output quantization error (~0.2% of a value that itself
    # carries ~0.4% bf16 matmul noise) is far inside the accuracy budget,
    # and it halves the output DMA.
    y_d = nc.dram_tensor("y", [NT, 128, H], BF16, kind="ExternalOutput")

    with TileContext(nc) as tc:
        with (
            tc.tile_pool(name="resident", bufs=1) as res,
            tc.tile_pool(name="wstream", bufs=3) as wstream,
            tc.tile_pool(name="tmp", bufs=2) as tmp,
            tc.tile_pool(name="ysb", bufs=2) as ysb,
            tc.tile_pool(name="psum", bufs=2, space="PSUM") as psum,
        ):
            xt_sb = res.tile([128, KH, C], BF16, tag="xt")
            act_sb = res.tile([128, NF, C], BF16, tag="act")
            w2_sb = res.tile([128, NF, H], BF16, tag="w2")
            wvt_sb = res.tile([128, NT], F32, tag="wvt")

            # All bulk streaming rides the two HWDGE rings (SP + ACT): HWDGE
            # issue cost is ~0.6us/DMA on an otherwise-idle sequencer, vs
            # ~1-2.4us of Q7 emission per SWDGE (gpsimd) DMA — at this
            # kernel's ~2.1us/f-tile cadence a gpsimd ring carrying a third
            # of the weight stream would be issue-rate-limited. The 16 SDMA
            # engines (and the ~358 GB/s HBM ceiling) are shared across
            # rings, so two rings lose no bandwidth. gpsimd carries only the
            # tiny wvt load. Order matters: xt feeds the very first matmul,
            # so its 8 chunks go 4/4 on both rings ahead of any weights.
            hwdge = [nc.sync, nc.scalar]
            for k in range(KH):
                hwdge[k % 2].dma_start(xt_sb[:, k, :], xt_d[k])
            nc.gpsimd.dma_start(wvt_sb[:], wvt_d[:])

            # PE warm-up: the HAM clock gate runs the PE at 1.2 GHz until it
            # has seen ~3.4us of sustained matmul activity. Burn that window
            # during the initial DMA wait with dependency-free garbage
            # matmuls (the PSUM bank is overwritten by the first real
            # accumulation's start=True). The warm tile is kept to a single
            # 128-wide block so its memset barely delays the first matmul.
            warm_sb = tmp.tile([128, 128], BF16, tag="warm")
            nc.vector.memset(warm_sb[:], 0.0)
            warm_ps = psum.tile([128, 128], F32, tag="g")
            n_warm = max(4, int(3600 // (128 / 1.2)))
            for _ in range(n_warm):
                nc.tensor.matmul(warm_ps[:], warm_sb[:],
                                 warm_sb[:], start=True, stop=True)

            # ---- phase 1: gT/uT = w1/w3 contractions over H (single
            # C-wide chunk per weight tile); act = silu(g)*u in bf16
            for f in range(NF):
                w1_sb = wstream.tile([128, KH, 128], BF16, tag="w1")
                w3_sb = wstream.tile([128, KH, 128], BF16, tag="w3")
                hwdge[f % 2].dma_start(w1_sb[:], w1t_d[f])
                hwdge[(f + 1) % 2].dma_start(w3_sb[:], w3t_d[f])
                # stream the phase-2 weights on the alternating ring, lagged
                # two iterations so the first w1/w3 fetches own the early
                # HBM bandwidth. Only the first 512-wide h-slice is part of
                # the phase-1 critical window: phase 2 touches w2[:, 512:]
                # a few us in, so that half is deferred below, shrinking the
                # DMA-bound phase-1 span by ~3.6 MB.
                if f >= 2:
                    hwdge[f % 2].dma_start(w2_sb[:, f - 2, 0:512],
                                           w2t_d[f - 2, :, 0:512])
                g_ps = psum.tile([128, C], F32, tag="g")
                u_ps = psum.tile([128, C], F32, tag="u")
                for k in range(KH):
                    nc.tensor.matmul(
                        g_ps[:], w1_sb[:, k, :], xt_sb[:, k, :],
                        start=(k == 0), stop=(k == KH - 1),
                    )
                for k in range(KH):
                    nc.tensor.matmul(
                        u_ps[:], w3_sb[:, k, :], xt_sb[:, k, :],
                        start=(k == 0), stop=(k == KH - 1),
                    )
                s_sb = tmp.tile([128, C], F32, tag="silu")
                nc.scalar.activation(
                    s_sb[:], g_ps[:], mybir.ActivationFunctionType.Silu
                )
                nc.vector.tensor_tensor(
                    act_sb[:, f, :], s_sb[:], u_ps[:],
                    mybir.AluOpType.mult,
                )
            # Tail of the w2 stream, emitted in phase 2's consumption order
            # (ring order = arrival order): phase 2's first f-steps touch
            # w2[0..3, 512:] within ~1us of the boundary, while the last two
            # hs0 tiles aren't read until ~5.5us in, and the rest of the
            # deferred hs1 slice is consumed paced at ~0.2us/f-tile, which
            # the two-ring supply sustains.
            for f in range(4):
                hwdge[f % 2].dma_start(w2_sb[:, f, 512:1024],
                                       w2t_d[f, :, 512:1024])
            for f in range(NF - 2, NF):
                hwdge[f % 2].dma_start(w2_sb[:, f, 0:512], w2t_d[f, :, 0:512])
            for f in range(4, NF):
                hwdge[f % 2].dma_start(w2_sb[:, f, 512:1024],
                                       w2t_d[f, :, 512:1024])

            # ---- phase 2: act-stationary. y[t, h] = sum_f act[f, t].T @
            # w2T[f, h]: the 128x128 act tile is the stationary operand
            # (112 Ldweights instead of 224) and w2 streams as the moving
            # operand in two 512-wide h-slices (one PSUM bank each). The
            # per-token combine weight is applied by the ACT engine as a
            # per-partition scale during the PSUM->SBUF copy, so the output
            # leaves in [token, hidden] orientation (no host transpose).
            for t in range(NT):
                y0_ps = psum.tile([128, 512], F32, tag="y0")
                y1_ps = psum.tile([128, 512], F32, tag="y1")
                for f in range(NF):
                    a_t = act_sb[:, f, t * 128:(t + 1) * 128]
                    nc.tensor.matmul(
                        y0_ps[:], a_t, w2_sb[:, f, 0:512],
                        start=(f == 0), stop=(f == NF - 1),
                    )
                    nc.tensor.matmul(
                        y1_ps[:], a_t, w2_sb[:, f, 512:1024],
                        start=(f == 0), stop=(f == NF - 1),
                    )
                y_sb = ysb.tile([128, H], BF16, tag="y")
                wv_col = wvt_sb[:, t:t + 1]
                # drain the two banks on different engines so they run in
                # parallel (matters for the final tile's exposed tail)
                nc.scalar.mul(y_sb[:, 0:512], y0_ps[:], wv_col)
                nc.vector.tensor_scalar_mul(y_sb[:, 512:1024], y1_ps[:],
                                            wv_col)
                if t == NT - 1:
                    # split the last tile across both HWDGE rings so its
                    # completion latencies overlap
                    nc.sync.dma_start(y_d[t, :, 0:512], y_sb[:, 0:512])
                    nc.scalar.dma_start(y_d[t, :, 512:1024], y_sb[:, 512:1024])
                else:
                    hwdge[t % 2].dma_start(y_d[t], y_sb[:])

    return nc


def kernel(hidden_states, gate_w, w1, w3, w2):
    x = np.ascontiguousarray(np.asarray(hidden_states, np.float32)).reshape(-1, H)
    gate_w = np.asarray(gate_w, np.float32)
    w1 = np.asarray(w1, np.float32)
    w3 = np.asarray(w3, np.float32)
    w2 = np.asarray(w2, np.float32)
    T = x.shape[0]

    topk_idx, topk_w = _route(x, gate_w)

    idx_e, wv_e = [], []
    for e in range(E):
        sel_t, sel_k = np.nonzero(topk_idx == e)
        idx_e.append(sel_t)
        wv_e.append(topk_w[sel_t, sel_k])
    maxT = max(len(i) for i in idx_e)
    # Device capacity: <=512 tokens per expert (single PSUM-bank-wide matmul
    # chunks; multiple of 128 for the phase-2 token tiles). Overflow slots of
    # overloaded experts run on the host below.
    C = max(128, min(_ceil_to(maxT, 128), 512))

    xbf = x.astype(NPBF16)
    in_maps = []
    for e in range(E):
        n = min(len(idx_e[e]), C)
        xg = np.zeros((C, H), NPBF16)
        xg[:n] = xbf[idx_e[e][:n]]
        xt = np.ascontiguousarray(xg.T).reshape(H // 128, 128, C)
        w1t = np.ascontiguousarray(
            w1[e].astype(NPBF16).reshape(F // 128, 128, H // 128, 128)
            .transpose(0, 3, 2, 1)
        )
        w3t = np.ascontiguousarray(
            w3[e].astype(NPBF16).reshape(F // 128, 128, H // 128, 128)
            .transpose(0, 3, 2, 1)
        )
        w2t = np.ascontiguousarray(w2[e].T.astype(NPBF16)).reshape(F // 128, 128, H)
        wv = np.zeros(C, np.float32)
        wv[:n] = wv_e[e][:n]
        # [128, NT]: partition p, column t -> combine weight of token 128t+p
        wvt = np.ascontiguousarray(wv.reshape(C // 128, 128).T)
        in_maps.append({"xt": xt, "w1t": w1t, "w3t": w3t, "w2t": w2t,
                        "wvt": wvt})

    nc = _build_bass(C)
    res = run_bass_kernel_spmd(nc, in_maps, core_ids=list(range(NCORES)))
    global last_results, last_in_maps, last_C
    last_results, last_in_maps, last_C = res, in_maps, C

    out = np.zeros((T, H), np.float32)
    for e in range(E):
        n = min(len(idx_e[e]), C)
        yt = res.results[e]["y"].reshape(C, H)   # [C, H] bf16, token-major
        out[idx_e[e][:n]] += yt[:n].astype(np.float32)
        if len(idx_e[e]) > C:
            # Capacity overflow: exact host-side SiLU MLP for the few
            # leftover token slots of this expert.
            ov = idx_e[e][C:]
            xo = x[ov]                              # [m, H] f32
            g = xo @ w1[e].T
            u = xo @ w3[e].T
            act = (g / (1.0 + np.exp(-g))) * u
            yo = act @ w2[e].T
            out[ov] += wv_e[e][C:, None] * yo
    return out.reshape(1, T, H).astype(np.float32)

